# revision 2
# baseline (speedup 1.0000x reference)
"""Trainium2 Bass kernel for BeliefTreeMemory GNN message passing, v2.

Strategy (8 NeuronCores, SPMD, one program):
  - Host permutes nodes into (core, tile) bins balancing edges per bin
    (<=1024 edges per 512-node tile) -> uniform padded layout, ~0 skew.
  - Row-major per-edge MLP: y1[e, dout] accumulated from stationary
    sT/tT/srh chunks vs moving W1 parts; silu via ACT table; aggregation
    P[dmid, v] += y1s_chunk^T @ S with fp16 indicator S built on DVE.
  - W2 folded into the GRU input weights on host (Wx = Wih @ W2), so
    gates read P directly; has_msg mask via rank-1 matmul on z gate.
  - Tiles processed in groups: all message/agg work for ~12 tiles, then
    all GRUs -> only 2 ACT table switches per group.
  - Pass-1 src AND tgt rows pre-gathered + pre-transposed on host (h0
    known). Pass-2: tgt via gpsimd ap_gather from resident f32 slab;
    src via 4-chunk-batched indirect DMA from bf16 h1full + PE transpose.
  - One bf16 AllGather of row-major h1 shards between passes.
  - Pass-2 output written in transposed (slab) layout; host untransposes.
"""

import sys
import numpy as np
import ml_dtypes

BF16 = ml_dtypes.bfloat16
F16 = np.float16

sys.path.insert(0, "/opt/trn_rl_repo")

N_NODES = 200000
N_EDGES = 400000
D = 128
N_CORES = 8
N_PASSES = 2
TN = 512
BIG = 30.0
FAKE_SILU = False   # sim lacks the Silu table
DEBUG_DUMP = False  # add y1s/P debug outputs (sim debugging)


class _Cfg:
    def __init__(self, n_nodes=N_NODES, n_edges=N_EDGES):
        assert n_nodes % N_CORES == 0
        self.N = n_nodes
        self.E = n_edges
        self.NSH = n_nodes // N_CORES
        self.NSHP = ((self.NSH + TN - 1) // TN) * TN
        self.NTILES = self.NSHP // TN


def _rup(x, m):
    return ((x + m - 1) // m) * m


def _groups(nt, maxg=13):
    ng = max(1, (nt + maxg - 1) // maxg)
    base = nt // ng
    rem = nt - base * ng
    out = []
    t0 = 0
    for i in range(ng):
        wd = base + (1 if i < rem else 0)
        out.append(list(range(t0, t0 + wd)))
        t0 += wd
    return out


def _balance(cfg, cnt):
    """Assign nodes to (core, tile, slot) balancing edge counts.

    Returns per-core slot lists of length NSHP (old node id or -1 pad),
    tile-major."""
    import heapq
    N, NSH, NT = cfg.N, cfg.NSH, cfg.NTILES
    order = np.argsort(-cnt, kind="stable")
    core_nodes = [[] for _ in range(N_CORES)]
    heap = [(0, c) for c in range(N_CORES)]
    heapq.heapify(heap)
    for n in order:
        while True:
            e, c = heapq.heappop(heap)
            if len(core_nodes[c]) < NSH:
                break
        core_nodes[c].append(n)
        heapq.heappush(heap, (e + int(cnt[n]), c))
    ecap = TN * 2
    node_of = []
    for c in range(N_CORES):
        nodes = core_nodes[c]
        nodes.sort(key=lambda n: -int(cnt[n]))
        bin_nodes = [[] for _ in range(NT)]
        bin_edges = [0] * NT
        h2 = [(0, t) for t in range(NT)]
        heapq.heapify(h2)
        for n in nodes:
            stash = []
            placed = False
            while h2:
                e, t = heapq.heappop(h2)
                if len(bin_nodes[t]) < TN and (e + int(cnt[n]) <= ecap):
                    bin_nodes[t].append(n)
                    bin_edges[t] = e + int(cnt[n])
                    heapq.heappush(h2, (bin_edges[t], t))
                    placed = True
                    break
                stash.append((e, t))
            for it in stash:
                heapq.heappush(h2, it)
            if not placed:
                cand = sorted((bin_edges[t], t) for t in range(NT)
                              if len(bin_nodes[t]) < TN)
                t = cand[0][1]
                bin_nodes[t].append(n)
                bin_edges[t] += int(cnt[n])
                # heap left stale for this t; rebuild
                h2 = [(bin_edges[tt], tt) for tt in range(NT)
                      if len(bin_nodes[tt]) < TN]
                heapq.heapify(h2)
        torder = np.argsort(-np.array(bin_edges), kind="stable")
        flat = []
        for t in torder:
            b = bin_nodes[t]
            flat.extend(b)
            flat.extend([-1] * (TN - len(b)))
        node_of.append(flat)
    return node_of


def _plan(cfg, inputs):
    N, NSH, NSHP, NT = cfg.N, cfg.NSH, cfg.NSHP, cfg.NTILES
    src = np.asarray(inputs["src"]).astype(np.int64)
    tgt = np.asarray(inputs["tgt"]).astype(np.int64)
    etype = np.asarray(inputs["etype"]).astype(np.int64)
    cred = np.asarray(inputs["cred"], np.float32)
    h0 = np.asarray(inputs["h"], np.float32)

    cnt = np.bincount(tgt, minlength=N).astype(np.int64)
    rec = (1.0 / np.maximum(cnt, 1)).astype(np.float32)

    node_of = _balance(cfg, cnt)
    coreof = np.full(N, -1, np.int32)
    iloc = np.full(N, -1, np.int32)
    for c in range(N_CORES):
        arr = np.array(node_of[c], np.int64)
        real = arr >= 0
        coreof[arr[real]] = c
        iloc[arr[real]] = np.nonzero(real)[0].astype(np.int32)

    e_core = coreof[tgt]
    e_tile = iloc[tgt] // TN

    counts = np.zeros((N_CORES, NT), np.int64)
    for c in range(N_CORES):
        m = e_core == c
        counts[c] = np.bincount(e_tile[m], minlength=NT)
    P = np.array([_rup(max(int(counts[:, t].max()), 1), 128)
                  for t in range(NT)], np.int64)
    E_pad = int(P.sum())
    nch = E_pad // 128
    offs = np.concatenate([[0], np.cumsum(P)]).astype(np.int64)
    # chunked-AllGather layout: group g covers node slots [s0, s1);
    # h1full row for (core c, slot i in group g) = 8*s0 + c*(s1-s0) + (i-s0)
    groups = _groups(NT)
    grp_of_tile = np.zeros(NT, np.int64)
    grp_bounds = []
    for gi, g in enumerate(groups):
        for t in g:
            grp_of_tile[t] = gi
        grp_bounds.append((g[0] * TN, (g[-1] + 1) * TN))

    def h1row(cvec, ivec):
        gi = grp_of_tile[ivec // TN]
        s0 = np.array([grp_bounds[x][0] for x in gi], np.int64)
        s1 = np.array([grp_bounds[x][1] for x in gi], np.int64)
        return N_CORES * s0 + cvec.astype(np.int64) * (s1 - s0) + (ivec - s0)

    per_core = []
    for c in range(N_CORES):
        SRCg = np.zeros(E_pad, np.int64)
        SRCo = np.zeros(E_pad, np.int64)
        TGTo = np.zeros(E_pad, np.int64)
        TLOC = np.zeros(E_pad, np.int64)
        TLR = np.full(E_pad, -1.0, np.float32)
        RECe = np.zeros(E_pad, np.float32)
        SRH = np.zeros((6, E_pad), np.float32)
        m = e_core == c
        eids = np.nonzero(m)[0]
        et = e_tile[eids]
        for t in range(NT):
            ids = eids[et == t]
            nv = len(ids)
            off = int(offs[t])
            sl = slice(off, off + nv)
            SRCo[sl] = src[ids]
            TGTo[sl] = tgt[ids]
            SRCg[sl] = h1row(coreof[src[ids]], iloc[src[ids]].astype(np.int64))
            tl = (iloc[tgt[ids]] - t * TN).astype(np.int64)
            TLOC[sl] = tl
            TLR[sl] = tl.astype(np.float32)
            RECe[sl] = rec[tgt[ids]]
            SRH[0, sl] = cred[src[ids]]
            SRH[1:5, sl] = np.eye(4, dtype=np.float32)[etype[ids]].T
            SRH[5, sl] = 1.0

        srcg_img = SRCg.reshape(nch, 128).T.astype(np.int32)
        tlr_img = np.ascontiguousarray(TLR.reshape(nch, 128).T)
        tlr2_img = np.ascontiguousarray(tlr_img - 256.0)
        rec_img = np.ascontiguousarray(RECe.reshape(nch, 128).T)
        apg = np.zeros((128, 8 * nch), np.int16)
        for ch in range(nch):
            u = TLOC[ch * 128:(ch + 1) * 128]
            blk = u.reshape(8, 16).T.astype(np.int16)
            apg[:, 8 * ch:8 * (ch + 1)] = np.tile(blk, (8, 1))
        sfeedT = np.ascontiguousarray(h0[SRCo].T.astype(BF16))
        tfeedT = np.ascontiguousarray(h0[TGTo].T.astype(BF16))

        arr = np.array(node_of[c], np.int64)
        real = arr >= 0
        hshT = np.zeros((D, NSHP), np.float32)
        hshT[:, real] = h0[arr[real]].T
        nomsg = np.full((1, NSHP), BIG, np.float32)
        nomsg[0, real] = BIG * (cnt[arr[real]] == 0)

        per_core.append(dict(
            srcg=np.ascontiguousarray(srcg_img),
            tlr=tlr_img, tlr2=tlr2_img, recg=rec_img, apg=apg,
            srhs=np.ascontiguousarray(SRH.astype(BF16)),
            sfeedT=sfeedT, tfeedT=tfeedT,
            hsh0T=hshT, nomsg=nomsg.astype(BF16)))

    W1 = np.asarray(inputs["W1"], np.float32)
    ee = np.asarray(inputs["edge_emb"], np.float32)
    W2 = np.asarray(inputs["W2"], np.float32)
    b2 = np.asarray(inputs["b2"], np.float32)
    Wih = np.asarray(inputs["Wih"], np.float32)
    Whh = np.asarray(inputs["Whh"], np.float32)
    bih = np.asarray(inputs["bih"], np.float32)
    bhh = np.asarray(inputs["bhh"], np.float32)
    Wx = Wih @ W2
    bi = bih + Wih @ b2
    shared = dict(
        Wr1s=np.ascontiguousarray(W1[:, :D].T).astype(BF16),
        Wr1t=np.ascontiguousarray(W1[:, D:2 * D].T).astype(BF16),
        Wr1a=np.ascontiguousarray(np.concatenate(
            [W1[:, 2 * D + 64][None, :],
             ee @ W1[:, 2 * D:2 * D + 64].T,
             np.asarray(inputs["b1"], np.float32)[None, :]], 0)).astype(BF16),
        b1col=np.asarray(inputs["b1"], np.float32)[:, None].copy(),
        WxT=np.ascontiguousarray(Wx.T).astype(BF16),
        WhhT=np.ascontiguousarray(Whh.T).astype(BF16),
        brcol=np.ascontiguousarray((bi[:D] + bhh[:D])[:, None]),
        bzcol=np.ascontiguousarray((bi[D:2 * D] + bhh[D:2 * D])[:, None]),
        bnhcol=np.ascontiguousarray(bhh[2 * D:][:, None]),
        bnicol=np.ascontiguousarray(bi[2 * D:][:, None]),
        ones1=np.ones((1, D), BF16),
        eyeb=np.eye(D, dtype=BF16),
        iotab=np.ascontiguousarray(
            np.tile(np.arange(256).astype(BF16), (128, 1))),
    )
    meta = dict(P=P, E_pad=E_pad, nch=nch, offs=offs, node_of=node_of)
    return meta, per_core, shared


def _build(cfg, meta):
    from concourse import bacc, tile, mybir
    import concourse.bass as bass

    nc = bacc.Bacc("TRN2", target_bir_lowering=False, debug=False,
                   num_devices=N_CORES)
    f32, i32, i16 = mybir.dt.float32, mybir.dt.int32, mybir.dt.int16
    bf16, fp16 = mybir.dt.bfloat16, mybir.dt.float16
    AF = mybir.ActivationFunctionType
    NSHP, NT = cfg.NSHP, cfg.NTILES
    P = meta["P"]
    nch = meta["nch"]
    E_pad = meta["E_pad"]
    offs = meta["offs"]
    PMAX = int(P.max())
    groups = _groups(NT)
    NGR = max(len(g) for g in groups)

    srcg = nc.dram_tensor("srcg", [128, nch], i32, kind="ExternalInput")
    tlr = nc.dram_tensor("tlr", [128, nch], f32, kind="ExternalInput")
    tlr2 = nc.dram_tensor("tlr2", [128, nch], f32, kind="ExternalInput")
    recg = nc.dram_tensor("recg", [128, nch], f32, kind="ExternalInput")
    apg = nc.dram_tensor("apg", [128, 8 * nch], i16, kind="ExternalInput")
    srhs = nc.dram_tensor("srhs", [6, E_pad], bf16, kind="ExternalInput")
    sfeedT = nc.dram_tensor("sfeedT", [128, E_pad], bf16,
                            kind="ExternalInput")
    tfeedT = nc.dram_tensor("tfeedT", [128, E_pad], bf16,
                            kind="ExternalInput")
    hsh0T = nc.dram_tensor("hsh0T", [D, NSHP], f32, kind="ExternalInput")
    nomsg = nc.dram_tensor("nomsg", [1, NSHP], bf16, kind="ExternalInput")
    wnames = dict(Wr1s=([D, D], bf16), Wr1t=([D, D], bf16),
                  Wr1a=([6, D], bf16), b1col=([D, 1], f32),
                  WxT=([D, 3 * D], bf16), WhhT=([D, 3 * D], bf16),
                  brcol=([D, 1], f32), bzcol=([D, 1], f32),
                  bnhcol=([D, 1], f32), bnicol=([D, 1], f32),
                  ones1=([1, D], bf16), eyeb=([D, D], bf16),
                  iotab=([128, 256], bf16))
    wt = {k: nc.dram_tensor(k, sh, dt, kind="ExternalInput")
          for k, (sh, dt) in wnames.items()}
    h_outT = nc.dram_tensor("h_outT", [D, NSHP], f32, kind="ExternalOutput")
    if DEBUG_DUMP:
        y1sdbg = nc.dram_tensor("y1sdbg", [E_pad, D], bf16,
                                kind="ExternalOutput")
        Pdbg = nc.dram_tensor("Pdbg", [D, NSHP], bf16, kind="ExternalOutput")
        h1dbg = nc.dram_tensor("h1dbg", [NSHP, D], bf16,
                               kind="ExternalOutput")
        gsrdbg = nc.dram_tensor("gsrdbg", [E_pad, D], bf16,
                                kind="ExternalOutput")
    h1rm = nc.dram_tensor("h1rm", [NSHP, D], bf16)
    h1full = nc.dram_tensor("h1full", [N_CORES * NSHP, D], bf16,
                            addr_space="Shared")

    with tile.TileContext(nc) as tc:
        with (
            tc.tile_pool(name="const", bufs=1) as cpool,
            tc.tile_pool(name="feed", bufs=2) as fpool,
            tc.tile_pool(name="blk", bufs=4) as bpool,
            tc.tile_pool(name="pb", bufs=NGR) as ppool,
            tc.tile_pool(name="gru", bufs=2) as gpool,
            tc.tile_pool(name="psY", bufs=2, space="PSUM") as psY,
            tc.tile_pool(name="psP", bufs=1, space="PSUM") as psP,
            tc.tile_pool(name="psG", bufs=1, space="PSUM") as psG,
        ):
            w = {}
            for k, (sh, dt) in wnames.items():
                w[k] = cpool.tile(sh, dt, tag=k, name=f"w_{k}")
                nc.sync.dma_start(out=w[k][:, :], in_=wt[k][:, :])
            slab = cpool.tile([D, NSHP], f32, tag="slab", name="slab")
            nc.sync.dma_start(out=slab[:, :], in_=hsh0T[:, :])
            srcg_sb = cpool.tile([128, nch], i32, tag="srcg", name="srcg_sb")
            nc.sync.dma_start(out=srcg_sb[:, :], in_=srcg[:, :])
            tlr_sb = cpool.tile([128, nch], f32, tag="tlr", name="tlr_sb")
            nc.sync.dma_start(out=tlr_sb[:, :], in_=tlr[:, :])
            tlr2_sb = cpool.tile([128, nch], f32, tag="tlr2",
                                 name="tlr2_sb")
            nc.sync.dma_start(out=tlr2_sb[:, :], in_=tlr2[:, :])
            rec_sb = cpool.tile([128, nch], f32, tag="rec", name="rec_sb")
            nc.sync.dma_start(out=rec_sb[:, :], in_=recg[:, :])
            apg_sb = cpool.tile([128, 8 * nch], i16, tag="apg", name="apg_sb")
            nc.sync.dma_start(out=apg_sb[:, :], in_=apg[:, :])

            for p in range(N_PASSES):
                for grp in groups:
                    pbs = {}
                    for t in grp:
                        off = int(offs[t])
                        tch = int(P[t]) // 128
                        ch0 = off // 128
                        cl, chh = TN * t, TN * (t + 1)
                        P_ps = psP.tile([128, TN], f32, tag="P",
                                        name=f"P_{p}_{t}")
                        sTt = fpool.tile([128, PMAX], bf16, tag="sT",
                                         name=f"sTt_{p}_{t}")
                        tTt = fpool.tile([128, PMAX], bf16, tag="tT",
                                         name=f"tTt_{p}_{t}")
                        if p == 0:
                            nc.sync.dma_start(
                                out=sTt[:, :int(P[t])],
                                in_=sfeedT[:, off:off + int(P[t])])
                            nc.sync.dma_start(
                                out=tTt[:, :int(P[t])],
                                in_=tfeedT[:, off:off + int(P[t])])
                        srh_t = fpool.tile([6, PMAX], bf16, tag="srh",
                                           name=f"srh_{p}_{t}")
                        nc.sync.dma_start(
                            out=srh_t[:, :int(P[t])],
                            in_=srhs[:, off:off + int(P[t])])
                        for b0 in range(0, tch, 4):
                            bw = min(4, tch - b0)
                            cb = ch0 + b0
                            y1 = psY.tile([128, 512], f32, tag="y1",
                                          name=f"y1_{p}_{cb}")
                            if p == 1:
                                gsr = bpool.tile([128, 512], bf16, tag="gsr",
                                                 name=f"gsr_{cb}")
                                for a in range(bw):
                                    nc.gpsimd.indirect_dma_start(
                                        out=gsr[:, 128 * a:128 * (a + 1)],
                                        out_offset=None,
                                        in_=h1full[:, :],
                                        in_offset=bass.IndirectOffsetOnAxis(
                                            ap=srcg_sb[:, cb + a:cb + a + 1],
                                            axis=0))
                                if DEBUG_DUMP:
                                    nc.sync.dma_start(
                                        out=gsrdbg[128 * cb:128 * (cb + bw), :]
                                        .rearrange("(a e) d -> e a d", e=128),
                                        in_=gsr[:, :128 * bw]
                                        .rearrange("e (a d) -> e a d", d=128))
                                ptr = psY.tile([128, 512], bf16, tag="y1",
                                               name=f"ptr_{cb}")
                                for a in range(bw):
                                    nc.tensor.transpose(
                                        ptr[:, 128 * a:128 * (a + 1)],
                                        gsr[:, 128 * a:128 * (a + 1)],
                                        w["eyeb"][:, :])
                                nc.vector.tensor_copy(
                                    sTt[:, 128 * b0:128 * (b0 + bw)],
                                    ptr[:, :128 * bw])
                                tTf = bpool.tile([128, 512], f32, tag="tTf",
                                                 name=f"tTf_{cb}", bufs=2)
                                for a in range(bw):
                                    ch = cb + a
                                    nc.gpsimd.ap_gather(
                                        tTf[:, 128 * a:128 * (a + 1)],
                                        slab[:, cl:chh],
                                        apg_sb[:, 8 * ch:8 * (ch + 1)],
                                        channels=128, num_elems=TN, d=1,
                                        num_idxs=128)
                                nc.vector.tensor_copy(
                                    tTt[:, 128 * b0:128 * (b0 + bw)],
                                    tTf[:, :128 * bw])
                            for a in range(bw):
                                sl = slice(128 * (b0 + a), 128 * (b0 + a + 1))
                                ysl = slice(128 * a, 128 * (a + 1))
                                nc.tensor.matmul(
                                    y1[:, ysl], sTt[:, sl], w["Wr1s"][:, :],
                                    start=True, stop=False)
                                nc.tensor.matmul(
                                    y1[:, ysl], tTt[:, sl], w["Wr1t"][:, :],
                                    start=False, stop=False)
                                nc.tensor.matmul(
                                    y1[:, ysl], srh_t[:, sl], w["Wr1a"][:, :],
                                    start=False, stop=True)
                            y1s = bpool.tile([128, 512], bf16, tag="y1s",
                                             name=f"y1s_{p}_{cb}")
                            if FAKE_SILU:
                                zb = bpool.tile([128, 512], f32, tag="zb",
                                                name=f"zb_{p}_{cb}")
                                nc.scalar.activation(
                                    zb[:, :128 * bw], y1[:, :128 * bw],
                                    AF.Identity)
                                sg = bpool.tile([128, 512], f32, tag="sg",
                                                name=f"sg_{p}_{cb}")
                                nc.scalar.activation(
                                    sg[:, :128 * bw], y1[:, :128 * bw],
                                    AF.Sigmoid)
                                nc.vector.tensor_mul(y1s[:, :128 * bw],
                                                     zb[:, :128 * bw],
                                                     sg[:, :128 * bw])
                            else:
                                nc.scalar.activation(
                                    y1s[:, :128 * bw], y1[:, :128 * bw],
                                    AF.Silu)
                            if DEBUG_DUMP and p == 0:
                                nc.sync.dma_start(
                                    out=y1sdbg[128 * cb:128 * (cb + bw), :]
                                    .rearrange("(a e) d -> e a d", e=128),
                                    in_=y1s[:, :128 * bw]
                                    .rearrange("e (a d) -> e a d", d=128))
                            for a in range(bw):
                                ch = cb + a
                                S = bpool.tile([128, TN], bf16, tag="S",
                                               name=f"S_{p}_{ch}")
                                nc.vector.tensor_scalar(
                                    out=S[:, 0:256], in0=w["iotab"][:, :],
                                    scalar1=tlr_sb[:, ch:ch + 1],
                                    scalar2=rec_sb[:, ch:ch + 1],
                                    op0=mybir.AluOpType.is_equal,
                                    op1=mybir.AluOpType.mult)
                                nc.vector.tensor_scalar(
                                    out=S[:, 256:512], in0=w["iotab"][:, :],
                                    scalar1=tlr2_sb[:, ch:ch + 1],
                                    scalar2=rec_sb[:, ch:ch + 1],
                                    op0=mybir.AluOpType.is_equal,
                                    op1=mybir.AluOpType.mult)
                                nc.tensor.matmul(
                                    P_ps[:, :],
                                    y1s[:, 128 * a:128 * (a + 1)], S[:, :],
                                    start=(b0 == 0 and a == 0),
                                    stop=(b0 + bw >= tch and a == bw - 1))
                        Pb = ppool.tile([128, TN], bf16, tag="Pb",
                                        name=f"Pb_{p}_{t}")
                        nc.vector.tensor_copy(Pb[:, :], P_ps[:, :])
                        pbs[t] = Pb
                        if DEBUG_DUMP and p == 0:
                            nc.sync.dma_start(
                                out=Pdbg[:, TN * t:TN * (t + 1)],
                                in_=Pb[:, :])
                    # ---- GRU burst for the group (sigmoid/tanh table),
                    # software-pipelined so the eye-accumulate matmul of
                    # tile t-1 never stalls the PE.

                    def gru_gates(t):
                        cl, chh = TN * t, TN * (t + 1)
                        Pb = pbs[t]
                        hTb = gpool.tile([128, TN], bf16, tag="hTb",
                                         name=f"hTb_{p}_{t}")
                        nc.vector.tensor_copy(hTb[:, :], slab[:, cl:chh])
                        pr = psG.tile([128, TN], f32, tag="prh", bufs=2,
                                      name=f"pr_{p}_{t}")
                        pz = psG.tile([128, TN], f32, tag="pz",
                                      name=f"pz_{p}_{t}")
                        pni = psG.tile([128, TN], f32, tag="pni", bufs=2,
                                       name=f"pni_{p}_{t}")
                        pnh = psG.tile([128, TN], f32, tag="prh", bufs=2,
                                       name=f"pnh_{p}_{t}")
                        nm_t = gpool.tile([1, TN], bf16, tag="nm",
                                          name=f"nm_{p}_{t}")
                        nc.sync.dma_start(out=nm_t[:, :],
                                          in_=nomsg[:, cl:chh])
                        nc.tensor.matmul(pr[:, :], w["WxT"][:, 0:D],
                                         Pb[:, :], start=True, stop=False)
                        nc.tensor.matmul(pr[:, :], w["WhhT"][:, 0:D],
                                         hTb[:, :], start=False, stop=True)
                        nc.tensor.matmul(pz[:, :], w["WxT"][:, D:2 * D],
                                         Pb[:, :], start=True, stop=False)
                        nc.tensor.matmul(pz[:, :], w["WhhT"][:, D:2 * D],
                                         hTb[:, :], start=False, stop=False)
                        nc.tensor.matmul(pz[:, :], w["ones1"][:, :],
                                         nm_t[:, :], start=False, stop=True)
                        nc.tensor.matmul(pni[:, :], w["WxT"][:, 2 * D:3 * D],
                                         Pb[:, :], start=True, stop=False)
                        nc.tensor.matmul(pnh[:, :], w["WhhT"][:, 2 * D:3 * D],
                                         hTb[:, :], start=True, stop=True)
                        return dict(t=t, cl=cl, chh=chh, pr=pr, pz=pz,
                                    pni=pni, pnh=pnh)

                    def gru_pre(st):
                        t = st["t"]
                        r_s = gpool.tile([128, TN], bf16, tag="r_s",
                                         name=f"r_{p}_{t}")
                        nc.scalar.activation(r_s[:, :], st["pr"][:, :],
                                             AF.Sigmoid,
                                             bias=w["brcol"][:, 0:1])
                        ghn = gpool.tile([128, TN], bf16, tag="ghn",
                                         name=f"ghn_{p}_{t}")
                        nc.scalar.activation(ghn[:, :], st["pnh"][:, :],
                                             AF.Identity,
                                             bias=w["bnhcol"][:, 0:1])
                        t1 = gpool.tile([128, TN], bf16, tag="t1",
                                        name=f"t1_{p}_{t}")
                        nc.vector.tensor_mul(t1[:, :], r_s[:, :], ghn[:, :])
                        st["t1"] = t1

                    def gru_eye(st):
                        nc.tensor.matmul(st["pni"][:, :], w["eyeb"][:, :],
                                         st["t1"][:, :],
                                         start=False, stop=True)

                    def gru_finish(st):
                        t = st["t"]
                        cl, chh = st["cl"], st["chh"]
                        z_s = gpool.tile([128, TN], f32, tag="z_s",
                                         name=f"z_{p}_{t}")
                        nc.scalar.activation(z_s[:, :], st["pz"][:, :],
                                             AF.Sigmoid,
                                             bias=w["bzcol"][:, 0:1])
                        n_s = gpool.tile([128, TN], f32, tag="n_s",
                                         name=f"n_{p}_{t}")
                        nc.scalar.activation(n_s[:, :], st["pni"][:, :],
                                             AF.Tanh,
                                             bias=w["bnicol"][:, 0:1])
                        d_s = gpool.tile([128, TN], f32, tag="d_s",
                                         name=f"d_{p}_{t}")
                        nc.vector.tensor_sub(d_s[:, :], slab[:, cl:chh],
                                             n_s[:, :])
                        zd = gpool.tile([128, TN], f32, tag="zd",
                                        name=f"zd_{p}_{t}")
                        nc.vector.tensor_mul(zd[:, :], z_s[:, :], d_s[:, :])
                        if p == 0:
                            nc.vector.tensor_add(slab[:, cl:chh], n_s[:, :],
                                                 zd[:, :])
                            hnb = gpool.tile([128, TN], bf16, tag="hnb",
                                             name=f"hnb_{t}")
                            nc.vector.tensor_copy(hnb[:, :], slab[:, cl:chh])
                            ptb = psP.tile([128, TN], bf16, tag="P",
                                           name=f"ptb_{t}")
                            for a in range(4):
                                nc.tensor.transpose(
                                    ptb[:, 128 * a:128 * (a + 1)],
                                    hnb[:, 128 * a:128 * (a + 1)],
                                    w["eyeb"][:, :])
                            rowb = gpool.tile([128, TN], bf16, tag="rowb",
                                              name=f"rowb_{t}")
                            nc.vector.tensor_copy(rowb[:, :], ptb[:, :])
                            nc.sync.dma_start(
                                out=h1rm[cl:chh, :]
                                .rearrange("(a q) d -> q a d", q=128),
                                in_=rowb[:, :]
                                .rearrange("q (a d) -> q a d", d=128))
                        else:
                            hn = gpool.tile([128, TN], f32, tag="hn",
                                            name=f"hn_{t}")
                            nc.vector.tensor_add(hn[:, :], n_s[:, :],
                                                 zd[:, :])
                            nc.sync.dma_start(out=h_outT[:, cl:chh],
                                              in_=hn[:, :])

                    prev = None
                    for t in grp:
                        if prev is not None:
                            gru_eye(prev)
                        st = gru_gates(t)
                        if prev is not None:
                            gru_finish(prev)
                        gru_pre(st)
                        prev = st
                    gru_eye(prev)
                    gru_finish(prev)
                    if p == 0:
                        s0, s1 = TN * grp[0], TN * (grp[-1] + 1)
                        nc.gpsimd.collective_compute(
                            "AllGather", mybir.AluOpType.bypass,
                            replica_groups=[list(range(N_CORES))],
                            ins=[h1rm[s0:s1, :]],
                            outs=[h1full[N_CORES * s0:N_CORES * s1, :]])
                if p == 0 and DEBUG_DUMP:
                    nc.sync.dma_start(out=h1dbg[:, :], in_=h1rm[:, :])
    import concourse.bacc as bacc_mod
    from concourse.hw_specs import get_activation_tables as _gat
    _orig_tabs = _gat(nc.m.arch)
    _keep = {"sigmoid_and_others", "silu_and_others"}
    _patched = {k: (v if k in _keep else set()) for k, v in _orig_tabs.items()}
    _saved = bacc_mod.get_activation_tables
    bacc_mod.get_activation_tables = lambda arch: _patched
    try:
        nc.compile()
    finally:
        bacc_mod.get_activation_tables = _saved
    return nc


def build_and_run(inputs, cfg=None, sim=False, trace=False, tmpdir=None):
    global FAKE_SILU
    cfg = cfg or _Cfg()
    meta, per_core, shared = _plan(cfg, inputs)
    FAKE_SILU = bool(sim)
    nc = _build(cfg, meta)
    maps = []
    for c in range(N_CORES):
        m = {k: np.ascontiguousarray(v) for k, v in per_core[c].items()}
        m.update({k: np.ascontiguousarray(v) for k, v in shared.items()})
        maps.append(m)

    def assemble(shards):
        # shards[c] = h_outT [D, NSHP]; undo transpose + permutation
        out = np.zeros((cfg.N, D), np.float32)
        for c in range(N_CORES):
            arr = np.array(meta["node_of"][c], np.int64)
            real = arr >= 0
            out[arr[real]] = shards[c][:, real].T
        return out

    if sim:
        from concourse.bass_interp import MultiCoreSim
        ms = MultiCoreSim(nc, num_cores=N_CORES, trace=False)
        for c in range(N_CORES):
            for k, v in maps[c].items():
                ms.cores[c].tensor(k)[:] = v
        ms.simulate(check_with_hw=False)
        shards = [np.array(ms.cores[c].tensor("h_outT"))
                  for c in range(N_CORES)]
        return assemble(shards), None
    from concourse import bass_utils
    res = bass_utils.run_bass_kernel_spmd(
        nc, maps, list(range(N_CORES)), trace=trace, tmpdir=tmpdir)
    shards = [res.results[c]["h_outT"] for c in range(N_CORES)]
    return assemble(shards), res


def kernel(**inputs):
    out, _ = build_and_run(inputs)
    return out.astype(np.float32)


# revision 3
# speedup vs baseline: 1.0136x; 1.0136x over previous
"""Trainium2 Bass kernel for BeliefTreeMemory GNN message passing, v2.

Strategy (8 NeuronCores, SPMD, one program):
  - Host permutes nodes into (core, tile) bins balancing edges per bin
    (<=1024 edges per 512-node tile) -> uniform padded layout, ~0 skew.
  - Row-major per-edge MLP: y1[e, dout] accumulated from stationary
    sT/tT/srh chunks vs moving W1 parts; silu via ACT table; aggregation
    P[dmid, v] += y1s_chunk^T @ S with fp16 indicator S built on DVE.
  - W2 folded into the GRU input weights on host (Wx = Wih @ W2), so
    gates read P directly; has_msg mask via rank-1 matmul on z gate.
  - Tiles processed in groups: all message/agg work for ~12 tiles, then
    all GRUs -> only 2 ACT table switches per group.
  - Pass-1 src AND tgt rows pre-gathered + pre-transposed on host (h0
    known). Pass-2: tgt via gpsimd ap_gather from resident f32 slab;
    src via per-chunk indirect DMA from bf16 h1full + PE transpose.
  - bf16 AllGather of h1, chunked per tile-group so it overlaps the
    pass-1 tail; h1full laid out group-major (host indexes accordingly).
  - GRU software-pipelined (eye-accumulate of tile t-1 issued between
    tile t's gate matmuls); silu decomposed as x*sigmoid(x) so every
    activation lives in one ACT table set (pinned at compile).
  - Pass-2 output written in transposed (slab) layout; host untransposes.
"""

import sys
import numpy as np
import ml_dtypes

BF16 = ml_dtypes.bfloat16
F16 = np.float16

sys.path.insert(0, "/opt/trn_rl_repo")

N_NODES = 200000
N_EDGES = 400000
D = 128
N_CORES = 8
N_PASSES = 2
TN = 512
BIG = 30.0
FAKE_SILU = False   # sim lacks the Silu table
DEBUG_DUMP = False  # add y1s/P debug outputs (sim debugging)


class _Cfg:
    def __init__(self, n_nodes=N_NODES, n_edges=N_EDGES):
        assert n_nodes % N_CORES == 0
        self.N = n_nodes
        self.E = n_edges
        self.NSH = n_nodes // N_CORES
        self.NSHP = ((self.NSH + TN - 1) // TN) * TN
        self.NTILES = self.NSHP // TN


def _rup(x, m):
    return ((x + m - 1) // m) * m


def _groups(nt, maxg=13):
    ng = max(1, (nt + maxg - 1) // maxg)
    base = nt // ng
    rem = nt - base * ng
    out = []
    t0 = 0
    for i in range(ng):
        wd = base + (1 if i < rem else 0)
        out.append(list(range(t0, t0 + wd)))
        t0 += wd
    return out


def _balance(cfg, cnt):
    """Assign nodes to (core, tile, slot) balancing edge counts.

    Returns per-core slot lists of length NSHP (old node id or -1 pad),
    tile-major."""
    import heapq
    N, NSH, NT = cfg.N, cfg.NSH, cfg.NTILES
    order = np.argsort(-cnt, kind="stable")
    core_nodes = [[] for _ in range(N_CORES)]
    heap = [(0, c) for c in range(N_CORES)]
    heapq.heapify(heap)
    for n in order:
        while True:
            e, c = heapq.heappop(heap)
            if len(core_nodes[c]) < NSH:
                break
        core_nodes[c].append(n)
        heapq.heappush(heap, (e + int(cnt[n]), c))
    ecap = TN * 2
    node_of = []
    for c in range(N_CORES):
        nodes = core_nodes[c]
        nodes.sort(key=lambda n: -int(cnt[n]))
        bin_nodes = [[] for _ in range(NT)]
        bin_edges = [0] * NT
        h2 = [(0, t) for t in range(NT)]
        heapq.heapify(h2)
        for n in nodes:
            stash = []
            placed = False
            while h2:
                e, t = heapq.heappop(h2)
                if len(bin_nodes[t]) < TN and (e + int(cnt[n]) <= ecap):
                    bin_nodes[t].append(n)
                    bin_edges[t] = e + int(cnt[n])
                    heapq.heappush(h2, (bin_edges[t], t))
                    placed = True
                    break
                stash.append((e, t))
            for it in stash:
                heapq.heappush(h2, it)
            if not placed:
                cand = sorted((bin_edges[t], t) for t in range(NT)
                              if len(bin_nodes[t]) < TN)
                t = cand[0][1]
                bin_nodes[t].append(n)
                bin_edges[t] += int(cnt[n])
                # heap left stale for this t; rebuild
                h2 = [(bin_edges[tt], tt) for tt in range(NT)
                      if len(bin_nodes[tt]) < TN]
                heapq.heapify(h2)
        torder = np.argsort(-np.array(bin_edges), kind="stable")
        flat = []
        for t in torder:
            b = bin_nodes[t]
            flat.extend(b)
            flat.extend([-1] * (TN - len(b)))
        node_of.append(flat)
    return node_of


def _plan(cfg, inputs):
    N, NSH, NSHP, NT = cfg.N, cfg.NSH, cfg.NSHP, cfg.NTILES
    src = np.asarray(inputs["src"]).astype(np.int64)
    tgt = np.asarray(inputs["tgt"]).astype(np.int64)
    etype = np.asarray(inputs["etype"]).astype(np.int64)
    cred = np.asarray(inputs["cred"], np.float32)
    h0 = np.asarray(inputs["h"], np.float32)

    cnt = np.bincount(tgt, minlength=N).astype(np.int64)
    rec = (1.0 / np.maximum(cnt, 1)).astype(np.float32)

    node_of = _balance(cfg, cnt)
    coreof = np.full(N, -1, np.int32)
    iloc = np.full(N, -1, np.int32)
    for c in range(N_CORES):
        arr = np.array(node_of[c], np.int64)
        real = arr >= 0
        coreof[arr[real]] = c
        iloc[arr[real]] = np.nonzero(real)[0].astype(np.int32)

    e_core = coreof[tgt]
    e_tile = iloc[tgt] // TN

    counts = np.zeros((N_CORES, NT), np.int64)
    for c in range(N_CORES):
        m = e_core == c
        counts[c] = np.bincount(e_tile[m], minlength=NT)
    P = np.array([_rup(max(int(counts[:, t].max()), 1), 128)
                  for t in range(NT)], np.int64)
    E_pad = int(P.sum())
    nch = E_pad // 128
    offs = np.concatenate([[0], np.cumsum(P)]).astype(np.int64)
    # chunked-AllGather layout: group g covers node slots [s0, s1);
    # h1full row for (core c, slot i in group g) = 8*s0 + c*(s1-s0) + (i-s0)
    groups = _groups(NT)
    grp_of_tile = np.zeros(NT, np.int64)
    grp_bounds = []
    for gi, g in enumerate(groups):
        for t in g:
            grp_of_tile[t] = gi
        grp_bounds.append((g[0] * TN, (g[-1] + 1) * TN))

    def h1row(cvec, ivec):
        gi = grp_of_tile[ivec // TN]
        s0 = np.array([grp_bounds[x][0] for x in gi], np.int64)
        s1 = np.array([grp_bounds[x][1] for x in gi], np.int64)
        return N_CORES * s0 + cvec.astype(np.int64) * (s1 - s0) + (ivec - s0)

    per_core = []
    for c in range(N_CORES):
        SRCg = np.zeros(E_pad, np.int64)
        SRCo = np.zeros(E_pad, np.int64)
        TGTo = np.zeros(E_pad, np.int64)
        TLOC = np.zeros(E_pad, np.int64)
        TLR = np.full(E_pad, -1.0, np.float32)
        RECe = np.zeros(E_pad, np.float32)
        SRH = np.zeros((6, E_pad), np.float32)
        m = e_core == c
        eids = np.nonzero(m)[0]
        et = e_tile[eids]
        for t in range(NT):
            ids = eids[et == t]
            nv = len(ids)
            off = int(offs[t])
            sl = slice(off, off + nv)
            SRCo[sl] = src[ids]
            TGTo[sl] = tgt[ids]
            SRCg[sl] = h1row(coreof[src[ids]], iloc[src[ids]].astype(np.int64))
            tl = (iloc[tgt[ids]] - t * TN).astype(np.int64)
            TLOC[sl] = tl
            TLR[sl] = tl.astype(np.float32)
            RECe[sl] = rec[tgt[ids]]
            SRH[0, sl] = cred[src[ids]]
            SRH[1:5, sl] = np.eye(4, dtype=np.float32)[etype[ids]].T
            SRH[5, sl] = 1.0

        srcg_img = SRCg.reshape(nch, 128).T.astype(np.int32)
        tlr_img = np.ascontiguousarray(TLR.reshape(nch, 128).T)
        rec_img = np.ascontiguousarray(RECe.reshape(nch, 128).T)
        apg = np.zeros((128, 8 * nch), np.int16)
        for ch in range(nch):
            u = TLOC[ch * 128:(ch + 1) * 128]
            blk = u.reshape(8, 16).T.astype(np.int16)
            apg[:, 8 * ch:8 * (ch + 1)] = np.tile(blk, (8, 1))
        sfeedT = np.ascontiguousarray(h0[SRCo].T.astype(BF16))
        tfeedT = np.ascontiguousarray(h0[TGTo].T.astype(BF16))

        arr = np.array(node_of[c], np.int64)
        real = arr >= 0
        hshT = np.zeros((D, NSHP), np.float32)
        hshT[:, real] = h0[arr[real]].T
        nomsg = np.full((1, NSHP), BIG, np.float32)
        nomsg[0, real] = BIG * (cnt[arr[real]] == 0)

        per_core.append(dict(
            srcg=np.ascontiguousarray(srcg_img),
            tlr=tlr_img, recg=rec_img, apg=apg,
            srhs=np.ascontiguousarray(SRH.astype(BF16)),
            sfeedT=sfeedT, tfeedT=tfeedT,
            hsh0T=hshT, nomsg=nomsg.astype(BF16)))

    W1 = np.asarray(inputs["W1"], np.float32)
    ee = np.asarray(inputs["edge_emb"], np.float32)
    W2 = np.asarray(inputs["W2"], np.float32)
    b2 = np.asarray(inputs["b2"], np.float32)
    Wih = np.asarray(inputs["Wih"], np.float32)
    Whh = np.asarray(inputs["Whh"], np.float32)
    bih = np.asarray(inputs["bih"], np.float32)
    bhh = np.asarray(inputs["bhh"], np.float32)
    Wx = Wih @ W2
    bi = bih + Wih @ b2
    shared = dict(
        Wr1s=np.ascontiguousarray(W1[:, :D].T).astype(BF16),
        Wr1t=np.ascontiguousarray(W1[:, D:2 * D].T).astype(BF16),
        Wr1a=np.ascontiguousarray(np.concatenate(
            [W1[:, 2 * D + 64][None, :],
             ee @ W1[:, 2 * D:2 * D + 64].T,
             np.asarray(inputs["b1"], np.float32)[None, :]], 0)).astype(BF16),
        b1col=np.asarray(inputs["b1"], np.float32)[:, None].copy(),
        WxT=np.ascontiguousarray(Wx.T).astype(BF16),
        WhhT=np.ascontiguousarray(Whh.T).astype(BF16),
        brcol=np.ascontiguousarray((bi[:D] + bhh[:D])[:, None]),
        bzcol=np.ascontiguousarray((bi[D:2 * D] + bhh[D:2 * D])[:, None]),
        bnhcol=np.ascontiguousarray(bhh[2 * D:][:, None]),
        bnicol=np.ascontiguousarray(bi[2 * D:][:, None]),
        ones1=np.ones((1, D), BF16),
        eyeb=np.eye(D, dtype=BF16),
        iota16=np.ascontiguousarray(
            np.tile(np.arange(TN, dtype=F16), (128, 1))),
    )
    meta = dict(P=P, E_pad=E_pad, nch=nch, offs=offs, node_of=node_of)
    return meta, per_core, shared


def _build(cfg, meta):
    from concourse import bacc, tile, mybir
    import concourse.bass as bass

    nc = bacc.Bacc("TRN2", target_bir_lowering=False, debug=False,
                   num_devices=N_CORES)
    f32, i32, i16 = mybir.dt.float32, mybir.dt.int32, mybir.dt.int16
    bf16, fp16 = mybir.dt.bfloat16, mybir.dt.float16
    AF = mybir.ActivationFunctionType
    NSHP, NT = cfg.NSHP, cfg.NTILES
    P = meta["P"]
    nch = meta["nch"]
    E_pad = meta["E_pad"]
    offs = meta["offs"]
    PMAX = int(P.max())
    groups = _groups(NT)
    NGR = max(len(g) for g in groups)

    srcg = nc.dram_tensor("srcg", [128, nch], i32, kind="ExternalInput")
    tlr = nc.dram_tensor("tlr", [128, nch], f32, kind="ExternalInput")
    recg = nc.dram_tensor("recg", [128, nch], f32, kind="ExternalInput")
    apg = nc.dram_tensor("apg", [128, 8 * nch], i16, kind="ExternalInput")
    srhs = nc.dram_tensor("srhs", [6, E_pad], bf16, kind="ExternalInput")
    sfeedT = nc.dram_tensor("sfeedT", [128, E_pad], bf16,
                            kind="ExternalInput")
    tfeedT = nc.dram_tensor("tfeedT", [128, E_pad], bf16,
                            kind="ExternalInput")
    hsh0T = nc.dram_tensor("hsh0T", [D, NSHP], f32, kind="ExternalInput")
    nomsg = nc.dram_tensor("nomsg", [1, NSHP], bf16, kind="ExternalInput")
    wnames = dict(Wr1s=([D, D], bf16), Wr1t=([D, D], bf16),
                  Wr1a=([6, D], bf16), b1col=([D, 1], f32),
                  WxT=([D, 3 * D], bf16), WhhT=([D, 3 * D], bf16),
                  brcol=([D, 1], f32), bzcol=([D, 1], f32),
                  bnhcol=([D, 1], f32), bnicol=([D, 1], f32),
                  ones1=([1, D], bf16), eyeb=([D, D], bf16),
                  iota16=([128, TN], fp16))
    wt = {k: nc.dram_tensor(k, sh, dt, kind="ExternalInput")
          for k, (sh, dt) in wnames.items()}
    h_outT = nc.dram_tensor("h_outT", [D, NSHP], f32, kind="ExternalOutput")
    if DEBUG_DUMP:
        y1sdbg = nc.dram_tensor("y1sdbg", [E_pad, D], bf16,
                                kind="ExternalOutput")
        Pdbg = nc.dram_tensor("Pdbg", [D, NSHP], bf16, kind="ExternalOutput")
        h1dbg = nc.dram_tensor("h1dbg", [NSHP, D], bf16,
                               kind="ExternalOutput")
        gsrdbg = nc.dram_tensor("gsrdbg", [E_pad, D], bf16,
                                kind="ExternalOutput")
    h1rm = nc.dram_tensor("h1rm", [NSHP, D], bf16)
    h1full = nc.dram_tensor("h1full", [N_CORES * NSHP, D], bf16,
                            addr_space="Shared")

    with tile.TileContext(nc) as tc:
        with (
            tc.tile_pool(name="const", bufs=1) as cpool,
            tc.tile_pool(name="feed", bufs=2) as fpool,
            tc.tile_pool(name="blk", bufs=4) as bpool,
            tc.tile_pool(name="pb", bufs=NGR) as ppool,
            tc.tile_pool(name="gru", bufs=2) as gpool,
            tc.tile_pool(name="psY", bufs=2, space="PSUM") as psY,
            tc.tile_pool(name="psP", bufs=1, space="PSUM") as psP,
            tc.tile_pool(name="psG", bufs=1, space="PSUM") as psG,
        ):
            w = {}
            for k, (sh, dt) in wnames.items():
                w[k] = cpool.tile(sh, dt, tag=k, name=f"w_{k}")
                nc.sync.dma_start(out=w[k][:, :], in_=wt[k][:, :])
            slab = cpool.tile([D, NSHP], f32, tag="slab", name="slab")
            nc.sync.dma_start(out=slab[:, :], in_=hsh0T[:, :])
            srcg_sb = cpool.tile([128, nch], i32, tag="srcg", name="srcg_sb")
            nc.sync.dma_start(out=srcg_sb[:, :], in_=srcg[:, :])
            tlr_sb = cpool.tile([128, nch], f32, tag="tlr", name="tlr_sb")
            nc.sync.dma_start(out=tlr_sb[:, :], in_=tlr[:, :])
            rec_sb = cpool.tile([128, nch], f32, tag="rec", name="rec_sb")
            nc.sync.dma_start(out=rec_sb[:, :], in_=recg[:, :])
            apg_sb = cpool.tile([128, 8 * nch], i16, tag="apg", name="apg_sb")
            nc.sync.dma_start(out=apg_sb[:, :], in_=apg[:, :])

            for p in range(N_PASSES):
                for grp in groups:
                    pbs = {}
                    for t in grp:
                        off = int(offs[t])
                        tch = int(P[t]) // 128
                        ch0 = off // 128
                        cl, chh = TN * t, TN * (t + 1)
                        P_ps = psP.tile([128, TN], f32, tag="P",
                                        name=f"P_{p}_{t}")
                        sTt = fpool.tile([128, PMAX], bf16, tag="sT",
                                         name=f"sTt_{p}_{t}")
                        tTt = fpool.tile([128, PMAX], bf16, tag="tT",
                                         name=f"tTt_{p}_{t}")
                        if p == 0:
                            nc.sync.dma_start(
                                out=sTt[:, :int(P[t])],
                                in_=sfeedT[:, off:off + int(P[t])])
                            nc.sync.dma_start(
                                out=tTt[:, :int(P[t])],
                                in_=tfeedT[:, off:off + int(P[t])])
                        srh_t = fpool.tile([6, PMAX], bf16, tag="srh",
                                           name=f"srh_{p}_{t}")
                        nc.sync.dma_start(
                            out=srh_t[:, :int(P[t])],
                            in_=srhs[:, off:off + int(P[t])])
                        for b0 in range(0, tch, 4):
                            bw = min(4, tch - b0)
                            cb = ch0 + b0
                            y1 = psY.tile([128, 512], f32, tag="y1",
                                          name=f"y1_{p}_{cb}")
                            if p == 1:
                                gsr = bpool.tile([128, 512], bf16, tag="gsr",
                                                 name=f"gsr_{cb}")
                                for a in range(bw):
                                    nc.gpsimd.indirect_dma_start(
                                        out=gsr[:, 128 * a:128 * (a + 1)],
                                        out_offset=None,
                                        in_=h1full[:, :],
                                        in_offset=bass.IndirectOffsetOnAxis(
                                            ap=srcg_sb[:, cb + a:cb + a + 1],
                                            axis=0))
                                if DEBUG_DUMP:
                                    nc.sync.dma_start(
                                        out=gsrdbg[128 * cb:128 * (cb + bw), :]
                                        .rearrange("(a e) d -> e a d", e=128),
                                        in_=gsr[:, :128 * bw]
                                        .rearrange("e (a d) -> e a d", d=128))
                                ptr = psY.tile([128, 512], bf16, tag="y1",
                                               name=f"ptr_{cb}")
                                for a in range(bw):
                                    nc.tensor.transpose(
                                        ptr[:, 128 * a:128 * (a + 1)],
                                        gsr[:, 128 * a:128 * (a + 1)],
                                        w["eyeb"][:, :])
                                nc.vector.tensor_copy(
                                    sTt[:, 128 * b0:128 * (b0 + bw)],
                                    ptr[:, :128 * bw])
                                tTf = bpool.tile([128, 512], f32, tag="tTf",
                                                 name=f"tTf_{cb}", bufs=2)
                                for a in range(bw):
                                    ch = cb + a
                                    nc.gpsimd.ap_gather(
                                        tTf[:, 128 * a:128 * (a + 1)],
                                        slab[:, cl:chh],
                                        apg_sb[:, 8 * ch:8 * (ch + 1)],
                                        channels=128, num_elems=TN, d=1,
                                        num_idxs=128)
                                nc.vector.tensor_copy(
                                    tTt[:, 128 * b0:128 * (b0 + bw)],
                                    tTf[:, :128 * bw])
                            for a in range(bw):
                                sl = slice(128 * (b0 + a), 128 * (b0 + a + 1))
                                ysl = slice(128 * a, 128 * (a + 1))
                                nc.tensor.matmul(
                                    y1[:, ysl], sTt[:, sl], w["Wr1s"][:, :],
                                    start=True, stop=False)
                                nc.tensor.matmul(
                                    y1[:, ysl], tTt[:, sl], w["Wr1t"][:, :],
                                    start=False, stop=False)
                                nc.tensor.matmul(
                                    y1[:, ysl], srh_t[:, sl], w["Wr1a"][:, :],
                                    start=False, stop=True)
                            y1s = bpool.tile([128, 512], bf16, tag="y1s",
                                             name=f"y1s_{p}_{cb}")
                            sg = bpool.tile([128, 512], f32, tag="sg",
                                            name=f"sg_{p}_{cb}")
                            nc.scalar.activation(
                                sg[:, :128 * bw], y1[:, :128 * bw],
                                AF.Sigmoid)
                            nc.vector.tensor_mul(y1s[:, :128 * bw],
                                                 y1[:, :128 * bw],
                                                 sg[:, :128 * bw])
                            if DEBUG_DUMP and p == 0:
                                nc.sync.dma_start(
                                    out=y1sdbg[128 * cb:128 * (cb + bw), :]
                                    .rearrange("(a e) d -> e a d", e=128),
                                    in_=y1s[:, :128 * bw]
                                    .rearrange("e (a d) -> e a d", d=128))
                            for a in range(bw):
                                ch = cb + a
                                S = bpool.tile([128, TN], bf16, tag="S",
                                               name=f"S_{p}_{ch}")
                                nc.vector.tensor_scalar(
                                    out=S[:, :], in0=w["iota16"][:, :],
                                    scalar1=tlr_sb[:, ch:ch + 1],
                                    scalar2=rec_sb[:, ch:ch + 1],
                                    op0=mybir.AluOpType.is_equal,
                                    op1=mybir.AluOpType.mult)
                                nc.tensor.matmul(
                                    P_ps[:, :],
                                    y1s[:, 128 * a:128 * (a + 1)], S[:, :],
                                    start=(b0 == 0 and a == 0),
                                    stop=(b0 + bw >= tch and a == bw - 1))
                        Pb = ppool.tile([128, TN], bf16, tag="Pb",
                                        name=f"Pb_{p}_{t}")
                        nc.scalar.activation(Pb[:, :], P_ps[:, :],
                                             AF.Identity)
                        pbs[t] = Pb
                        if DEBUG_DUMP and p == 0:
                            nc.sync.dma_start(
                                out=Pdbg[:, TN * t:TN * (t + 1)],
                                in_=Pb[:, :])
                    # ---- GRU burst for the group (sigmoid/tanh table),
                    # software-pipelined so the eye-accumulate matmul of
                    # tile t-1 never stalls the PE.

                    def gru_gates(t):
                        cl, chh = TN * t, TN * (t + 1)
                        Pb = pbs[t]
                        hTb = gpool.tile([128, TN], bf16, tag="hTb",
                                         name=f"hTb_{p}_{t}")
                        nc.vector.tensor_copy(hTb[:, :], slab[:, cl:chh])
                        pr = psG.tile([128, TN], f32, tag="prh", bufs=2,
                                      name=f"pr_{p}_{t}")
                        pz = psG.tile([128, TN], f32, tag="pz",
                                      name=f"pz_{p}_{t}")
                        pni = psG.tile([128, TN], f32, tag="pni", bufs=2,
                                       name=f"pni_{p}_{t}")
                        pnh = psG.tile([128, TN], f32, tag="prh", bufs=2,
                                       name=f"pnh_{p}_{t}")
                        nm_t = gpool.tile([1, TN], bf16, tag="nm",
                                          name=f"nm_{p}_{t}")
                        nc.sync.dma_start(out=nm_t[:, :],
                                          in_=nomsg[:, cl:chh])
                        nc.tensor.matmul(pr[:, :], w["WxT"][:, 0:D],
                                         Pb[:, :], start=True, stop=False)
                        nc.tensor.matmul(pr[:, :], w["WhhT"][:, 0:D],
                                         hTb[:, :], start=False, stop=True)
                        nc.tensor.matmul(pz[:, :], w["WxT"][:, D:2 * D],
                                         Pb[:, :], start=True, stop=False)
                        nc.tensor.matmul(pz[:, :], w["WhhT"][:, D:2 * D],
                                         hTb[:, :], start=False, stop=False)
                        nc.tensor.matmul(pz[:, :], w["ones1"][:, :],
                                         nm_t[:, :], start=False, stop=True)
                        nc.tensor.matmul(pni[:, :], w["WxT"][:, 2 * D:3 * D],
                                         Pb[:, :], start=True, stop=False)
                        nc.tensor.matmul(pnh[:, :], w["WhhT"][:, 2 * D:3 * D],
                                         hTb[:, :], start=True, stop=True)
                        return dict(t=t, cl=cl, chh=chh, pr=pr, pz=pz,
                                    pni=pni, pnh=pnh)

                    def gru_pre(st):
                        t = st["t"]
                        r_s = gpool.tile([128, TN], bf16, tag="r_s",
                                         name=f"r_{p}_{t}")
                        nc.scalar.activation(r_s[:, :], st["pr"][:, :],
                                             AF.Sigmoid,
                                             bias=w["brcol"][:, 0:1])
                        ghn = gpool.tile([128, TN], bf16, tag="ghn",
                                         name=f"ghn_{p}_{t}")
                        nc.scalar.activation(ghn[:, :], st["pnh"][:, :],
                                             AF.Identity,
                                             bias=w["bnhcol"][:, 0:1])
                        t1 = gpool.tile([128, TN], bf16, tag="t1",
                                        name=f"t1_{p}_{t}")
                        nc.vector.tensor_mul(t1[:, :], r_s[:, :], ghn[:, :])
                        st["t1"] = t1

                    def gru_eye(st):
                        nc.tensor.matmul(st["pni"][:, :], w["eyeb"][:, :],
                                         st["t1"][:, :],
                                         start=False, stop=True)

                    def gru_finish(st):
                        t = st["t"]
                        cl, chh = st["cl"], st["chh"]
                        z_s = gpool.tile([128, TN], f32, tag="z_s",
                                         name=f"z_{p}_{t}")
                        nc.scalar.activation(z_s[:, :], st["pz"][:, :],
                                             AF.Sigmoid,
                                             bias=w["bzcol"][:, 0:1])
                        n_s = gpool.tile([128, TN], f32, tag="n_s",
                                         name=f"n_{p}_{t}")
                        nc.scalar.activation(n_s[:, :], st["pni"][:, :],
                                             AF.Tanh,
                                             bias=w["bnicol"][:, 0:1])
                        d_s = gpool.tile([128, TN], f32, tag="d_s",
                                         name=f"d_{p}_{t}")
                        nc.vector.tensor_sub(d_s[:, :], slab[:, cl:chh],
                                             n_s[:, :])
                        zd = gpool.tile([128, TN], f32, tag="zd",
                                        name=f"zd_{p}_{t}")
                        nc.vector.tensor_mul(zd[:, :], z_s[:, :], d_s[:, :])
                        if p == 0:
                            nc.vector.tensor_add(slab[:, cl:chh], n_s[:, :],
                                                 zd[:, :])
                            hnb = gpool.tile([128, TN], bf16, tag="hnb",
                                             name=f"hnb_{t}")
                            nc.vector.tensor_copy(hnb[:, :], slab[:, cl:chh])
                            ptb = psP.tile([128, TN], bf16, tag="P",
                                           name=f"ptb_{t}")
                            for a in range(4):
                                nc.tensor.transpose(
                                    ptb[:, 128 * a:128 * (a + 1)],
                                    hnb[:, 128 * a:128 * (a + 1)],
                                    w["eyeb"][:, :])
                            rowb = gpool.tile([128, TN], bf16, tag="rowb",
                                              name=f"rowb_{t}")
                            nc.vector.tensor_copy(rowb[:, :], ptb[:, :])
                            nc.sync.dma_start(
                                out=h1rm[cl:chh, :]
                                .rearrange("(a q) d -> q a d", q=128),
                                in_=rowb[:, :]
                                .rearrange("q (a d) -> q a d", d=128))
                        else:
                            hn = gpool.tile([128, TN], f32, tag="hn",
                                            name=f"hn_{t}")
                            nc.vector.tensor_add(hn[:, :], n_s[:, :],
                                                 zd[:, :])
                            nc.sync.dma_start(out=h_outT[:, cl:chh],
                                              in_=hn[:, :])

                    prev = None
                    for t in grp:
                        if prev is not None:
                            gru_eye(prev)
                        st = gru_gates(t)
                        if prev is not None:
                            gru_finish(prev)
                        gru_pre(st)
                        prev = st
                    gru_eye(prev)
                    gru_finish(prev)
                    if p == 0:
                        s0, s1 = TN * grp[0], TN * (grp[-1] + 1)
                        nc.gpsimd.collective_compute(
                            "AllGather", mybir.AluOpType.bypass,
                            replica_groups=[list(range(N_CORES))],
                            ins=[h1rm[s0:s1, :]],
                            outs=[h1full[N_CORES * s0:N_CORES * s1, :]])
                if p == 0 and DEBUG_DUMP:
                    nc.sync.dma_start(out=h1dbg[:, :], in_=h1rm[:, :])
    import concourse.bacc as bacc_mod
    from concourse.hw_specs import get_activation_tables as _gat
    _orig_tabs = _gat(nc.m.arch)
    _keep = {"sigmoid_and_others"}
    _patched = {k: (v if k in _keep else set()) for k, v in _orig_tabs.items()}
    _saved = bacc_mod.get_activation_tables
    bacc_mod.get_activation_tables = lambda arch: _patched
    try:
        nc.compile()
    finally:
        bacc_mod.get_activation_tables = _saved
    return nc


def build_and_run(inputs, cfg=None, sim=False, trace=False, tmpdir=None):
    global FAKE_SILU
    cfg = cfg or _Cfg()
    meta, per_core, shared = _plan(cfg, inputs)
    FAKE_SILU = bool(sim)
    nc = _build(cfg, meta)
    maps = []
    for c in range(N_CORES):
        m = {k: np.ascontiguousarray(v) for k, v in per_core[c].items()}
        m.update({k: np.ascontiguousarray(v) for k, v in shared.items()})
        maps.append(m)

    def assemble(shards):
        # shards[c] = h_outT [D, NSHP]; undo transpose + permutation
        out = np.zeros((cfg.N, D), np.float32)
        for c in range(N_CORES):
            arr = np.array(meta["node_of"][c], np.int64)
            real = arr >= 0
            out[arr[real]] = shards[c][:, real].T
        return out

    if sim:
        from concourse.bass_interp import MultiCoreSim
        ms = MultiCoreSim(nc, num_cores=N_CORES, trace=False)
        for c in range(N_CORES):
            for k, v in maps[c].items():
                ms.cores[c].tensor(k)[:] = v
        ms.simulate(check_with_hw=False)
        shards = [np.array(ms.cores[c].tensor("h_outT"))
                  for c in range(N_CORES)]
        return assemble(shards), None
    from concourse import bass_utils
    res = bass_utils.run_bass_kernel_spmd(
        nc, maps, list(range(N_CORES)), trace=trace, tmpdir=tmpdir)
    shards = [res.results[c]["h_outT"] for c in range(N_CORES)]
    return assemble(shards), res


def kernel(**inputs):
    out, _ = build_and_run(inputs)
    return out.astype(np.float32)
